# revision 1
# baseline (speedup 1.0000x reference)
"""Trainium2 Bass kernel for nn_Loss_2 (weighted BCE + index-gathered CE mean).

Data-parallel over 8 NeuronCores: each core processes 8 of the 64 batches,
computes per-partition partial sums on-chip, host sums 8x[128,1] partials and
divides by B*S.

Per-core program (tokens laid out [NT, 128, Tp] contiguous):
  LnC  = Ln(comb)                          (ScalarE, bf16)
  idxg = y_comb + (1-ys)*64                (pushes ys==0 tokens out of [0,20))
  D    = idxg_bcast - iota_class           (DVE, bf16; ==0 exactly at gathered class)
  cce_p = sum((D==0) * LnC)                (DVE scalar_tensor_tensor + accum_out)
  b1_p  = sum((ys*-W1) * Ln(ps))           (DVE scalar_tensor_tensor + accum_out)
  b0_p  = sum(((1-ys)*-W0) * Ln(1-ps))     (DVE scalar_tensor_tensor + accum_out)
  acc  += b1_p + b0_p - cce_p
"""

import sys

if '/opt/trn_rl_repo' not in sys.path:
    sys.path.insert(0, '/opt/trn_rl_repo')

import numpy as np

import concourse.bass as bass
import concourse.bacc as bacc
import concourse.tile as tile
import concourse.mybir as mybir
from concourse.bass_utils import run_bass_kernel_spmd

F32 = mybir.dt.float32
BF16 = mybir.dt.bfloat16
I32 = mybir.dt.int32
I16 = mybir.dt.int16

B, S, C = 64, 16384, 20
W0, W1 = 0.51, 19.05
BIG = 64.0
P = 128
N_CORES = 8
Tp = 256                       # tokens per partition per tile
NT = (B // N_CORES) * S // (P * Tp)  # 4 tiles per core


def _build(NT, Tp, comb_bufs=2):
    FREE = Tp * C
    nc = bacc.Bacc("TRN2", target_bir_lowering=False, debug=False)

    comb_d = nc.dram_tensor("comb", [NT, P, FREE], F32, kind="ExternalInput").ap()
    idxg_d = nc.dram_tensor("idxg", [NT, P, Tp], F32, kind="ExternalInput").ap()
    ps_d = nc.dram_tensor("ps", [NT, P, Tp], F32, kind="ExternalInput").ap()
    ys_d = nc.dram_tensor("ys", [NT, P, Tp], F32, kind="ExternalInput").ap()
    out_d = nc.dram_tensor("out", [P, 1], F32, kind="ExternalOutput").ap()

    with tile.TileContext(nc) as tc:
        with (
            tc.tile_pool(name="const", bufs=1) as const_pool,
            tc.tile_pool(name="comb", bufs=comb_bufs) as comb_pool,
            tc.tile_pool(name="big", bufs=2) as big_pool,
            tc.tile_pool(name="small", bufs=3) as small_pool,
        ):
            iota_t = const_pool.tile([P, FREE], I16)
            nc.gpsimd.iota(iota_t[:], pattern=[[0, Tp], [1, C]], base=0,
                           channel_multiplier=0)
            iota_v = iota_t[:].rearrange("p (t c) -> p t c", c=C)

            partsA = const_pool.tile([P, 2 * NT], F32)
            partsB = const_pool.tile([P, NT], F32)

            for i in range(NT):
                comb_t = comb_pool.tile([P, FREE], F32, tag="comb")
                nc.sync.dma_start(comb_t[:], comb_d[i])
                idxg = small_pool.tile([P, Tp], F32, tag="idxg")
                nc.sync.dma_start(idxg[:], idxg_d[i])
                ps_t = small_pool.tile([P, Tp], F32, tag="ps")
                nc.sync.dma_start(ps_t[:], ps_d[i])
                ys_t = small_pool.tile([P, Tp], F32, tag="ys")
                nc.sync.dma_start(ys_t[:], ys_d[i])

                lnc = big_pool.tile([P, FREE], BF16, tag="lnc")
                nc.scalar.activation(lnc[:], comb_t[:], mybir.ActivationFunctionType.Ln)

                idxg_b = idxg[:].rearrange("p (t o) -> p t o", o=1)

                mask = big_pool.tile([P, FREE], BF16, tag="mask")
                mask_v = mask[:].rearrange("p (t c) -> p t c", c=C)
                b_iota, b_idxg = bass.broadcast_tensor_aps(iota_v, idxg_b)
                nc.vector.tensor_tensor(mask_v, b_iota, b_idxg,
                                        mybir.AluOpType.is_equal)

                if False:
                    # DVE-only path: fused mult+sum on VectorE
                    nc.vector.scalar_tensor_tensor(
                        mask[:], mask[:], 1.0, lnc[:],
                        op0=mybir.AluOpType.mult, op1=mybir.AluOpType.mult,
                        accum_out=partsB[:, i:i + 1],
                    )
                else:
                    # split path: 2x bf16 multiply on DVE, sum on ScalarE
                    prod = big_pool.tile([P, FREE], BF16, tag="prod")
                    nc.vector.tensor_tensor(prod[:], mask[:], lnc[:],
                                            mybir.AluOpType.mult)
                    nc.scalar.activation(prod[:], prod[:],
                                         mybir.ActivationFunctionType.Copy,
                                         accum_out=partsB[:, i:i + 1])

                lps = small_pool.tile([P, Tp], F32, tag="lps")
                nc.scalar.activation(lps[:], ps_t[:], mybir.ActivationFunctionType.Ln)
                l1m = small_pool.tile([P, Tp], F32, tag="l1m")
                nc.scalar.activation(l1m[:], ps_t[:], mybir.ActivationFunctionType.Ln,
                                     bias=1.0, scale=-1.0)

                nc.vector.scalar_tensor_tensor(
                    lps[:], ys_t[:], -W1, lps[:],
                    op0=mybir.AluOpType.mult, op1=mybir.AluOpType.mult,
                    accum_out=partsA[:, 2 * i:2 * i + 1],
                )

                ys1m = small_pool.tile([P, Tp], F32, tag="ys1m")
                nc.vector.tensor_scalar(ys1m[:], ys_t[:], -1.0, 1.0,
                                        mybir.AluOpType.mult, mybir.AluOpType.add)
                nc.vector.scalar_tensor_tensor(
                    l1m[:], ys1m[:], -W0, l1m[:],
                    op0=mybir.AluOpType.mult, op1=mybir.AluOpType.mult,
                    accum_out=partsA[:, 2 * i + 1:2 * i + 2],
                )

            rA = const_pool.tile([P, 1], F32)
            nc.vector.tensor_reduce(rA[:], partsA[:], axis=mybir.AxisListType.X,
                                    op=mybir.AluOpType.add)
            rB = const_pool.tile([P, 1], F32)
            nc.vector.tensor_reduce(rB[:], partsB[:], axis=mybir.AxisListType.X,
                                    op=mybir.AluOpType.add)
            total = const_pool.tile([P, 1], F32)
            nc.vector.tensor_tensor(total[:], rA[:], rB[:], mybir.AluOpType.subtract)

            nc.sync.dma_start(out_d[:], total[:])

    nc.compile()
    return nc


_NC_CACHE = {}
IOTAC = np.ascontiguousarray(
    np.broadcast_to(np.tile(np.arange(C, dtype=np.int16), Tp), (P, Tp * C)))


def make_in_maps(y_pred_stroke, y_pred_comb, y_stroke, y_comb):
    y_pred_stroke = np.asarray(y_pred_stroke, dtype=np.float32)
    y_pred_comb = np.asarray(y_pred_comb, dtype=np.float32)
    y_stroke = np.asarray(y_stroke, dtype=np.float32)
    y_comb = np.asarray(y_comb)
    FREE = Tp * C
    Bc = B // N_CORES
    in_maps = []
    for c in range(N_CORES):
        sl = slice(c * Bc, (c + 1) * Bc)
        in_maps.append({
            "comb": np.ascontiguousarray(y_pred_comb[sl]).reshape(NT, P, FREE),
            "idxg": (np.ascontiguousarray(y_comb[sl]).astype(np.float32)
                     + (1.0 - np.ascontiguousarray(y_stroke[sl])[..., 0]) * BIG
                     ).reshape(NT, P, Tp),
            "ps": np.ascontiguousarray(y_pred_stroke[sl]).reshape(NT, P, Tp),
            "ys": np.ascontiguousarray(y_stroke[sl]).reshape(NT, P, Tp),
        })
    return in_maps


def kernel(y_pred_stroke, y_pred_comb, y_stroke, y_comb):
    key = (NT, Tp)
    if key not in _NC_CACHE:
        _NC_CACHE[key] = _build(NT, Tp)
    nc = _NC_CACHE[key]
    in_maps = make_in_maps(y_pred_stroke, y_pred_comb, y_stroke, y_comb)
    res = run_bass_kernel_spmd(nc, in_maps, list(range(N_CORES)))
    total = 0.0
    for r in res.results:
        total += r["out"].astype(np.float64).sum()
    return np.asarray([total / (B * S)], dtype=np.float32)



# revision 2
# speedup vs baseline: 1.4919x; 1.4919x over previous
"""Trainium2 Bass kernel for nn_Loss_2 (weighted BCE + index-gathered CE mean).

Data-parallel over 8 NeuronCores: each core processes 8 of the 64 batches.
All streams are bf16 (halves HBM traffic vs f32); per-partition partial sums
are accumulated in f32 on-chip, host does the final f64 reduction.

Per-core program, class-major tiles comb[P, C, Tp] (C=20 classes):
  lnc  = Ln(comb)                          (ScalarE, bf16 -> bf16)
  mask = (iota_c == idxg)                  (DVE TT is_equal, bf16, 2x)
  B_i  = sum(mask * lnc)                   (DVE scalar_tensor_tensor, accum f32)
  lq   = Ln(q), Q_i = sum(lq)              (ScalarE + accum_out)
  ysg  = (idxg >= 0)                       (DVE tensor_scalar, 4x)
  G_i  = sum(ysg * lq)                     (DVE stt accum)
Host: loss = (-W1*G - W0*(Q-G) - B) / (B*S)
  where idxg = y_comb if ys==1 else -1 (never matches a class -> mask row 0).
"""

import sys

if '/opt/trn_rl_repo' not in sys.path:
    sys.path.insert(0, '/opt/trn_rl_repo')

import numpy as np
import ml_dtypes

import concourse.bass as bass
import concourse.bacc as bacc
import concourse.tile as tile
import concourse.mybir as mybir
from concourse.bass_utils import run_bass_kernel_spmd

F32 = mybir.dt.float32
BF16 = mybir.dt.bfloat16
BF16_NP = ml_dtypes.bfloat16

B, S, C = 64, 16384, 20
W0, W1 = 0.51, 19.05
P = 128
N_CORES = 8
Tp = 256                        # tokens per partition per tile
NT = (B // N_CORES) * S // (P * Tp)   # tiles per core
IW = 64                         # iota inner period (dense run length)
ALU = mybir.AluOpType
AF = mybir.ActivationFunctionType


def _build(NT, Tp):
    FREE = Tp * C
    nc = bacc.Bacc("TRN2", target_bir_lowering=False, debug=False)

    comb_d = nc.dram_tensor("comb", [NT, P, FREE], BF16, kind="ExternalInput").ap()
    idxg_d = nc.dram_tensor("idxg", [NT, P, Tp], BF16, kind="ExternalInput").ap()
    q_d = nc.dram_tensor("q", [NT, P, Tp], BF16, kind="ExternalInput").ap()
    iota_d = nc.dram_tensor("iota", [P, C * IW], BF16, kind="ExternalInput").ap()
    out_d = nc.dram_tensor("out", [P, 3 * NT], F32, kind="ExternalOutput").ap()

    with tile.TileContext(nc) as tc:
        with (
            tc.tile_pool(name="const", bufs=1) as const_pool,
            tc.tile_pool(name="comb", bufs=2) as comb_pool,
            tc.tile_pool(name="lnc", bufs=2) as lnc_pool,
            tc.tile_pool(name="mask", bufs=2) as mask_pool,
            tc.tile_pool(name="small", bufs=3) as small_pool,
        ):
            # warm the natural_log activation table while first DMAs run
            warm = const_pool.tile([P, 1], BF16)
            nc.vector.memset(warm[:], 1.0)
            nc.scalar.activation(warm[:], warm[:], AF.Ln)

            iota_t = const_pool.tile([P, C * IW], BF16)
            nc.sync.dma_start(iota_t[:], iota_d[:])
            # [P, C, 1, IW] : class id c, dense inner run of IW
            iota_v = iota_t[:].rearrange("p (c o t) -> p c o t", c=C, o=1)

            parts = const_pool.tile([P, 3 * NT], F32)

            for i in range(NT):
                comb_t = comb_pool.tile([P, FREE], BF16, tag="comb")
                nc.sync.dma_start(comb_t[:], comb_d[i])
                idxg_t = small_pool.tile([P, Tp], BF16, tag="idxg")
                nc.sync.dma_start(idxg_t[:], idxg_d[i])
                q_t = small_pool.tile([P, Tp], BF16, tag="q")
                nc.sync.dma_start(q_t[:], q_d[i])

                lnc = lnc_pool.tile([P, FREE], BF16, tag="lnc")
                nc.scalar.activation(lnc[:], comb_t[:], AF.Ln)

                mask = mask_pool.tile([P, FREE], BF16, tag="mask")
                mask_v = mask[:].rearrange("p (c r t) -> p c r t", c=C, t=IW)
                idxg_v = idxg_t[:].rearrange("p (o r t) -> p o r t", o=1, t=IW)
                b_iota, b_idxg = bass.broadcast_tensor_aps(iota_v, idxg_v)
                nc.vector.tensor_tensor(mask_v, b_iota, b_idxg, ALU.is_equal)

                # B_i = sum(mask * lnc)   (in-place over mask, f32 accumulate)
                nc.vector.scalar_tensor_tensor(
                    mask[:], mask[:], 1.0, lnc[:],
                    op0=ALU.mult, op1=ALU.mult,
                    accum_out=parts[:, i:i + 1],
                )

                # Q_i = sum(ln q)
                lq = small_pool.tile([P, Tp], BF16, tag="lq")
                nc.scalar.activation(lq[:], q_t[:], AF.Ln,
                                     accum_out=parts[:, NT + i:NT + i + 1])

                # G_i = sum((idxg >= 0) * ln q)
                ysg = small_pool.tile([P, Tp], BF16, tag="ysg")
                nc.vector.tensor_scalar(ysg[:], idxg_t[:], 0.0, 1.0,
                                        ALU.is_ge, ALU.mult)
                scr = small_pool.tile([P, Tp], BF16, tag="scr")
                nc.vector.scalar_tensor_tensor(
                    scr[:], ysg[:], 1.0, lq[:],
                    op0=ALU.mult, op1=ALU.mult,
                    accum_out=parts[:, 2 * NT + i:2 * NT + i + 1],
                )

            nc.sync.dma_start(out_d[:], parts[:])

    nc.compile()
    return nc


_NC_CACHE = {}


def make_in_maps(y_pred_stroke, y_pred_comb, y_stroke, y_comb):
    y_pred_stroke = np.asarray(y_pred_stroke, dtype=np.float32)
    y_pred_comb = np.asarray(y_pred_comb, dtype=np.float32)
    y_stroke = np.asarray(y_stroke, dtype=np.float32)
    y_comb = np.asarray(y_comb)
    FREE = Tp * C
    Bc = B // N_CORES
    iota = np.ascontiguousarray(np.broadcast_to(
        np.repeat(np.arange(C, dtype=np.float32), IW).astype(BF16_NP),
        (P, C * IW)))
    in_maps = []
    for c in range(N_CORES):
        sl = slice(c * Bc, (c + 1) * Bc)
        ys = np.ascontiguousarray(y_stroke[sl])[..., 0].reshape(-1)
        ps = np.ascontiguousarray(y_pred_stroke[sl])[..., 0].reshape(-1)
        yc = np.ascontiguousarray(y_comb[sl]).reshape(-1)
        comb = (np.ascontiguousarray(y_pred_comb[sl])
                .reshape(NT, P, Tp, C)
                .transpose(0, 1, 3, 2)
                .reshape(NT, P, FREE))
        idxg = np.where(ys > 0.5, yc.astype(np.float32), -1.0)
        q = np.where(ys > 0.5, ps, 1.0 - ps)
        in_maps.append({
            "comb": np.ascontiguousarray(comb).astype(BF16_NP),
            "idxg": idxg.reshape(NT, P, Tp).astype(BF16_NP),
            "q": q.reshape(NT, P, Tp).astype(BF16_NP),
            "iota": iota,
        })
    return in_maps


def kernel(y_pred_stroke, y_pred_comb, y_stroke, y_comb):
    key = (NT, Tp)
    if key not in _NC_CACHE:
        _NC_CACHE[key] = _build(NT, Tp)
    nc = _NC_CACHE[key]
    in_maps = make_in_maps(y_pred_stroke, y_pred_comb, y_stroke, y_comb)
    res = run_bass_kernel_spmd(nc, in_maps, list(range(N_CORES)))
    total = 0.0
    for r in res.results:
        parts = r["out"].astype(np.float64)   # [P, 3*NT]
        Bs = parts[:, 0:NT].sum()
        Qs = parts[:, NT:2 * NT].sum()
        Gs = parts[:, 2 * NT:3 * NT].sum()
        total += -W1 * Gs - W0 * (Qs - Gs) - Bs
    return np.asarray([total / (B * S)], dtype=np.float32)


# revision 11
# speedup vs baseline: 1.9606x; 1.3142x over previous
"""Trainium2 Bass kernel for nn_Loss_2 (weighted BCE + index-gathered CE mean).

Data-parallel over 8 NeuronCores: each core processes 8 of the 64 batches.
The comb stream is fp8-e4m3 (quarter of f32 HBM traffic), partial sums are
f32 in PSUM, host does the final f64 weighted reduction.

The comb stream carries 22 "classes" per token (class-major [P, 22, Tp]):
  classes 0..19 : max(y_pred_comb, 2^-9)        (fp8 clamp, keeps ln finite)
  class  20     : ys ? 1 : (1-ps)   -> ln = (1-ys)*ln(1-ps)
  class  21     : ys ? ps : 1       -> ln = ys*ln(ps)
Per tile (Tp tokens/partition):
  lnc  = Ln(comb_ext)                     (ScalarE, fp8 -> bf16, 1 pass)
  mask = (iota_c == idxg), classes 0..19  (DVE TT is_equal bf16, 2x mode)
  prod = mask * lnc[0:20]                 (DVE TT mult bf16, 2x mode)
  PSUM A += colsum(prod)                  (TensorE ones-matmul, 10 chunks)
  PSUM B += colsum(lnc[20]); C += colsum(lnc[21])   (TensorE, 1 chunk each)
with idxg = y_comb if ys==1 else 20 (never matches -> mask row 0).
Host: loss = -(sum(A) + W0*sum(B) + W1*sum(C)) / (B*S)
"""

import sys

if '/opt/trn_rl_repo' not in sys.path:
    sys.path.insert(0, '/opt/trn_rl_repo')

import numpy as np
import ml_dtypes

import concourse.bass as bass
import concourse.bacc as bacc
import concourse.tile as tile
import concourse.mybir as mybir
from concourse.bass_utils import run_bass_kernel_spmd

F32 = mybir.dt.float32
BF16 = mybir.dt.bfloat16
FP8 = mybir.dt.float8e4
BF16_NP = ml_dtypes.bfloat16
FP8_NP = ml_dtypes.float8_e4m3fn

B, S, C = 64, 16384, 20
CE = C + 2                      # extended classes: +(1-ps)-gated, +ps-gated
W0, W1 = 0.51, 19.05
P = 128
N_CORES = 8
Tp = 256                        # tokens per partition per tile
NT = (B // N_CORES) * S // (P * Tp)   # tiles per core
IW = 64                         # iota inner period (dense run length)
AUXW = C * IW + NT * Tp         # iota block + all idxg tiles
MM = 512                        # matmul moving-free chunk (= psum bank f32)
ALU = mybir.AluOpType
AF = mybir.ActivationFunctionType


def _build(NT, Tp):
    FREE = Tp * CE              # full extended width
    CW = Tp * C                 # comb-classes width
    nc = bacc.Bacc("TRN2", target_bir_lowering=False, debug=False)

    comb_d = nc.dram_tensor("comb", [NT, P, FREE], FP8, kind="ExternalInput").ap()
    aux_d = nc.dram_tensor("aux", [P, AUXW], BF16, kind="ExternalInput").ap()
    out_d = nc.dram_tensor("out", [1, MM + 2 * Tp], F32, kind="ExternalOutput").ap()

    with tile.TileContext(nc) as tc:
        with (
            tc.tile_pool(name="const", bufs=1) as const_pool,
            tc.tile_pool(name="comb", bufs=1) as comb_pool,
            tc.tile_pool(name="lnc", bufs=2) as lnc_pool,
            tc.tile_pool(name="mask", bufs=2) as mask_pool,
            tc.tile_pool(name="prod", bufs=2) as prod_pool,
            tc.tile_pool(name="psum", bufs=1,
                         space=bass.MemorySpace.PSUM) as psum_pool,
        ):
            # warm the natural_log activation table while first DMAs run
            warm = const_pool.tile([P, 1], BF16)
            nc.vector.memset(warm[:], 1.0)
            nc.scalar.activation(warm[:], warm[:], AF.Ln)

            ones = const_pool.tile([P, 1], BF16)
            nc.vector.memset(ones[:], 1.0)

            # iota + all idxg tiles in one small DMA, ahead of the comb stream
            aux_t = const_pool.tile([P, AUXW], BF16)
            nc.sync.dma_start(aux_t[:], aux_d[:])
            iota_v = aux_t[:, 0:C * IW].rearrange("p (c o t) -> p c o t",
                                                  c=C, o=1)

            comb_ts = []
            for i in range(NT):
                comb_t = comb_pool.tile([P, FREE], FP8, tag=f"comb{i}")
                nc.sync.dma_start(comb_t[:], comb_d[i])
                comb_ts.append(comb_t)

            pA = psum_pool.tile([1, MM], F32, tag="pA")
            pB = psum_pool.tile([1, Tp], F32, tag="pB")
            pC = psum_pool.tile([1, Tp], F32, tag="pC")

            for i in range(NT):
                comb_t = comb_ts[i]
                off = C * IW + i * Tp
                idxg = aux_t[:, off:off + Tp]
                idxg_v = idxg.rearrange("p (o r t) -> p o r t", o=1, t=IW)

                lnc = lnc_pool.tile([P, FREE], BF16, tag="lnc")
                nc.scalar.activation(lnc[:], comb_t[:], AF.Ln)

                mask = mask_pool.tile([P, CW], BF16, tag="mask")
                mask_v = mask[:].rearrange("p (c r t) -> p c r t", c=C, t=IW)
                b_iota, b_idxg = bass.broadcast_tensor_aps(iota_v, idxg_v)
                nc.vector.tensor_tensor(mask_v, b_iota, b_idxg, ALU.is_equal)

                prod = prod_pool.tile([P, CW], BF16, tag="prod")
                nc.vector.tensor_tensor(prod[:], mask[:], lnc[:, 0:CW],
                                        ALU.mult)

                first, last = (i == 0), (i == NT - 1)
                for c in range(CW // MM):
                    nc.tensor.matmul(pA[:], ones[:],
                                     prod[:, c * MM:(c + 1) * MM],
                                     start=(first and c == 0),
                                     stop=(last and c == CW // MM - 1))
                nc.tensor.matmul(pB[:], ones[:], lnc[:, CW:CW + Tp],
                                 start=first, stop=last)
                nc.tensor.matmul(pC[:], ones[:], lnc[:, CW + Tp:FREE],
                                 start=first, stop=last)

            res_t = const_pool.tile([1, MM + 2 * Tp], F32)
            nc.scalar.copy(res_t[:, 0:MM], pA[:])
            nc.scalar.copy(res_t[:, MM:MM + Tp], pB[:])
            nc.scalar.copy(res_t[:, MM + Tp:MM + 2 * Tp], pC[:])
            nc.sync.dma_start(out_d[:], res_t[:])

    nc.compile()
    return nc


_NC_CACHE = {}


def make_in_maps(y_pred_stroke, y_pred_comb, y_stroke, y_comb):
    y_pred_stroke = np.asarray(y_pred_stroke, dtype=np.float32)
    y_pred_comb = np.asarray(y_pred_comb, dtype=np.float32)
    y_stroke = np.asarray(y_stroke, dtype=np.float32)
    y_comb = np.asarray(y_comb)
    FREE = Tp * CE
    Bc = B // N_CORES
    iota = np.repeat(np.arange(C, dtype=np.float32), IW)
    in_maps = []
    for c in range(N_CORES):
        sl = slice(c * Bc, (c + 1) * Bc)
        ys = np.ascontiguousarray(y_stroke[sl])[..., 0].reshape(-1)
        ps = np.ascontiguousarray(y_pred_stroke[sl])[..., 0].reshape(-1)
        yc = np.ascontiguousarray(y_comb[sl]).reshape(-1)
        pos = ys > 0.5
        comb = (np.maximum(np.ascontiguousarray(y_pred_comb[sl]), 2.0 ** -9)
                .reshape(NT, P, Tp, C)
                .transpose(0, 1, 3, 2))                     # [NT, P, C, Tp]
        q0 = np.where(pos, 1.0, 1.0 - ps).reshape(NT, P, 1, Tp)
        q1 = np.where(pos, ps, 1.0).reshape(NT, P, 1, Tp)
        comb_ext = np.concatenate([comb, q0, q1], axis=2).reshape(NT, P, FREE)
        idxg = np.where(pos, yc.astype(np.float32), 20.0)
        aux = np.empty((P, AUXW), dtype=np.float32)
        aux[:, 0:C * IW] = iota[None, :]
        aux[:, C * IW:] = (idxg.reshape(NT, P, Tp)
                           .transpose(1, 0, 2).reshape(P, NT * Tp))
        in_maps.append({
            "comb": np.ascontiguousarray(comb_ext).astype(FP8_NP),
            "aux": aux.astype(BF16_NP),
        })
    return in_maps


def kernel(y_pred_stroke, y_pred_comb, y_stroke, y_comb):
    key = (NT, Tp)
    if key not in _NC_CACHE:
        _NC_CACHE[key] = _build(NT, Tp)
    nc = _NC_CACHE[key]
    in_maps = make_in_maps(y_pred_stroke, y_pred_comb, y_stroke, y_comb)
    res = run_bass_kernel_spmd(nc, in_maps, list(range(N_CORES)))
    total = 0.0
    for r in res.results:
        o = r["out"].astype(np.float64).reshape(-1)
        total += (o[0:MM].sum() + W0 * o[MM:MM + Tp].sum()
                  + W1 * o[MM + Tp:].sum())
    return np.asarray([-total / (B * S)], dtype=np.float32)


# revision 14
# speedup vs baseline: 1.9656x; 1.0025x over previous
"""Trainium2 Bass kernel for nn_Loss_2 (weighted BCE + index-gathered CE mean).

Data-parallel over 8 NeuronCores: each core processes 8 of the 64 batches.
The comb stream is fp8-e4m3 (quarter of f32 HBM traffic), partial sums are
f32 in PSUM, host does the final f64 weighted reduction.

The comb stream carries 22 "classes" per token (class-major [P, 22, Tp]):
  classes 0..19 : max(y_pred_comb, 2^-9)        (fp8 clamp, keeps ln finite)
  class  20     : ys ? 1 : (1-ps)   -> ln = (1-ys)*ln(1-ps)
  class  21     : ys ? ps : 1       -> ln = ys*ln(ps)
Per tile (Tp tokens/partition):
  lnc  = Ln(comb_ext)                     (ScalarE, fp8 -> bf16, 1 pass)
  mask = (iota_c == idxg), classes 0..19  (DVE TT is_equal bf16, 2x mode)
  prod = mask * lnc[0:20]                 (DVE TT mult bf16, 2x mode)
  PSUM A += colsum(prod)                  (TensorE ones-matmul, 10 chunks)
  PSUM B += colsum(lnc[20]); C += colsum(lnc[21])   (TensorE, 1 chunk each)
with idxg = y_comb if ys==1 else 20 (never matches -> mask row 0).
Host: loss = -(sum(A) + W0*sum(B) + W1*sum(C)) / (B*S)
"""

import sys

if '/opt/trn_rl_repo' not in sys.path:
    sys.path.insert(0, '/opt/trn_rl_repo')

import numpy as np
import ml_dtypes

import concourse.bass as bass
import concourse.bacc as bacc
import concourse.tile as tile
import concourse.mybir as mybir
from concourse.bass_utils import run_bass_kernel_spmd

F32 = mybir.dt.float32
BF16 = mybir.dt.bfloat16
FP8 = mybir.dt.float8e4
BF16_NP = ml_dtypes.bfloat16
FP8_NP = ml_dtypes.float8_e4m3fn

B, S, C = 64, 16384, 20
CE = C + 2                      # extended classes: +(1-ps)-gated, +ps-gated
W0, W1 = 0.51, 19.05
P = 128
N_CORES = 8
Tp = 128                        # tokens per partition per tile
NT = (B // N_CORES) * S // (P * Tp)   # tiles per core
IW = 64                         # iota inner period (dense run length)
AUXW = C * IW + NT * Tp         # iota block + all idxg tiles
MM = 512                        # matmul moving-free chunk (= psum bank f32)
ALU = mybir.AluOpType
AF = mybir.ActivationFunctionType


def _build(NT, Tp):
    FREE = Tp * CE              # full extended width
    CW = Tp * C                 # comb-classes width
    nc = bacc.Bacc("TRN2", target_bir_lowering=False, debug=False)

    comb_d = nc.dram_tensor("comb", [NT, P, FREE], FP8, kind="ExternalInput").ap()
    aux_d = nc.dram_tensor("aux", [P, AUXW], BF16, kind="ExternalInput").ap()
    out_d = nc.dram_tensor("out", [1, MM + 2 * Tp], F32, kind="ExternalOutput").ap()

    with tile.TileContext(nc) as tc:
        with (
            tc.tile_pool(name="const", bufs=1) as const_pool,
            tc.tile_pool(name="comb", bufs=1) as comb_pool,
            tc.tile_pool(name="lnc", bufs=2) as lnc_pool,
            tc.tile_pool(name="mask", bufs=2) as mask_pool,
            tc.tile_pool(name="prod", bufs=2) as prod_pool,
            tc.tile_pool(name="psum", bufs=1,
                         space=bass.MemorySpace.PSUM) as psum_pool,
        ):
            # warm the natural_log activation table while first DMAs run
            warm = const_pool.tile([P, 1], BF16)
            nc.vector.memset(warm[:], 1.0)
            nc.scalar.activation(warm[:], warm[:], AF.Ln)

            ones = const_pool.tile([P, 1], BF16)
            nc.vector.memset(ones[:], 1.0)

            # comb0 leads the queue (it gates the ActE Ln chain), then the
            # small aux (iota + all idxg), then the rest of the comb stream.
            comb_t0 = comb_pool.tile([P, FREE], FP8, tag="comb0")
            comb_ts = [comb_t0]
            nc.sync.dma_start(comb_t0[:], comb_d[0])

            aux_t = const_pool.tile([P, AUXW], BF16)
            nc.sync.dma_start(aux_t[:], aux_d[:])
            iota_v = aux_t[:, 0:C * IW].rearrange("p (c o t) -> p c o t",
                                                  c=C, o=1)

            for i in range(1, NT):
                comb_t = comb_pool.tile([P, FREE], FP8, tag=f"comb{i}")
                nc.sync.dma_start(comb_t[:], comb_d[i])
                comb_ts.append(comb_t)

            pA = psum_pool.tile([1, MM], F32, tag="pA")
            pB = psum_pool.tile([1, Tp], F32, tag="pB")
            pC = psum_pool.tile([1, Tp], F32, tag="pC")

            for i in range(NT):
                comb_t = comb_ts[i]
                off = C * IW + i * Tp
                idxg = aux_t[:, off:off + Tp]
                idxg_v = idxg.rearrange("p (o r t) -> p o r t", o=1, t=IW)

                lnc = lnc_pool.tile([P, FREE], BF16, tag="lnc")
                nc.scalar.activation(lnc[:], comb_t[:], AF.Ln)

                mask = mask_pool.tile([P, CW], BF16, tag="mask")
                mask_v = mask[:].rearrange("p (c r t) -> p c r t", c=C, t=IW)
                b_iota, b_idxg = bass.broadcast_tensor_aps(iota_v, idxg_v)
                nc.vector.tensor_tensor(mask_v, b_iota, b_idxg, ALU.is_equal)

                prod = prod_pool.tile([P, CW], BF16, tag="prod")
                nc.vector.tensor_tensor(prod[:], mask[:], lnc[:, 0:CW],
                                        ALU.mult)

                first, last = (i == 0), (i == NT - 1)
                for c in range(CW // MM):
                    nc.tensor.matmul(pA[:], ones[:],
                                     prod[:, c * MM:(c + 1) * MM],
                                     start=(first and c == 0),
                                     stop=(last and c == CW // MM - 1))
                nc.tensor.matmul(pB[:], ones[:], lnc[:, CW:CW + Tp],
                                 start=first, stop=last)
                nc.tensor.matmul(pC[:], ones[:], lnc[:, CW + Tp:FREE],
                                 start=first, stop=last)

            res_t = const_pool.tile([1, MM + 2 * Tp], F32)
            nc.scalar.copy(res_t[:, 0:MM], pA[:])
            nc.scalar.copy(res_t[:, MM:MM + Tp], pB[:])
            nc.scalar.copy(res_t[:, MM + Tp:MM + 2 * Tp], pC[:])
            nc.sync.dma_start(out_d[:], res_t[:])

    nc.compile()
    return nc


_NC_CACHE = {}


def make_in_maps(y_pred_stroke, y_pred_comb, y_stroke, y_comb):
    y_pred_stroke = np.asarray(y_pred_stroke, dtype=np.float32)
    y_pred_comb = np.asarray(y_pred_comb, dtype=np.float32)
    y_stroke = np.asarray(y_stroke, dtype=np.float32)
    y_comb = np.asarray(y_comb)
    FREE = Tp * CE
    Bc = B // N_CORES
    iota = np.repeat(np.arange(C, dtype=np.float32), IW)
    in_maps = []
    for c in range(N_CORES):
        sl = slice(c * Bc, (c + 1) * Bc)
        ys = np.ascontiguousarray(y_stroke[sl])[..., 0].reshape(-1)
        ps = np.ascontiguousarray(y_pred_stroke[sl])[..., 0].reshape(-1)
        yc = np.ascontiguousarray(y_comb[sl]).reshape(-1)
        pos = ys > 0.5
        comb = (np.maximum(np.ascontiguousarray(y_pred_comb[sl]), 2.0 ** -9)
                .reshape(NT, P, Tp, C)
                .transpose(0, 1, 3, 2))                     # [NT, P, C, Tp]
        q0 = np.where(pos, 1.0, 1.0 - ps).reshape(NT, P, 1, Tp)
        q1 = np.where(pos, ps, 1.0).reshape(NT, P, 1, Tp)
        comb_ext = np.concatenate([comb, q0, q1], axis=2).reshape(NT, P, FREE)
        idxg = np.where(pos, yc.astype(np.float32), 20.0)
        aux = np.empty((P, AUXW), dtype=np.float32)
        aux[:, 0:C * IW] = iota[None, :]
        aux[:, C * IW:] = (idxg.reshape(NT, P, Tp)
                           .transpose(1, 0, 2).reshape(P, NT * Tp))
        in_maps.append({
            "comb": np.ascontiguousarray(comb_ext).astype(FP8_NP),
            "aux": aux.astype(BF16_NP),
        })
    return in_maps


def kernel(y_pred_stroke, y_pred_comb, y_stroke, y_comb):
    key = (NT, Tp)
    if key not in _NC_CACHE:
        _NC_CACHE[key] = _build(NT, Tp)
    nc = _NC_CACHE[key]
    in_maps = make_in_maps(y_pred_stroke, y_pred_comb, y_stroke, y_comb)
    res = run_bass_kernel_spmd(nc, in_maps, list(range(N_CORES)))
    total = 0.0
    for r in res.results:
        o = r["out"].astype(np.float64).reshape(-1)
        total += (o[0:MM].sum() + W0 * o[MM:MM + Tp].sum()
                  + W1 * o[MM + Tp:].sum())
    return np.asarray([-total / (B * S)], dtype=np.float32)


# revision 18
# speedup vs baseline: 2.0088x; 1.0220x over previous
"""Trainium2 Bass kernel for nn_Loss_2 (weighted BCE + index-gathered CE mean).

Data-parallel over 8 NeuronCores: each core processes 8 of the 64 batches.
The comb stream is fp8-e4m3 (quarter of f32 HBM traffic), partial sums are
f32 in PSUM, host does the final f64 weighted reduction.

The comb stream carries 22 "classes" per token (class-major [P, 22, Tp]):
  classes 0..19 : max(y_pred_comb, 2^-9)        (fp8 clamp, keeps ln finite)
  class  20     : ys ? 1 : (1-ps)   -> ln = (1-ys)*ln(1-ps)
  class  21     : ys ? ps : 1       -> ln = ys*ln(ps)
Per tile (Tp tokens/partition):
  lnc  = Ln(comb_ext)                     (ScalarE, fp8 -> bf16, 1 pass)
  mask = (iota_c == idxg), classes 0..19  (DVE TT is_equal bf16, 2x mode)
  prod = mask * lnc[0:20]                 (DVE TT mult bf16, 2x mode)
  PSUM A += colsum(prod)                  (TensorE ones-matmul, 10 chunks)
  PSUM B += colsum(lnc[20]); C += colsum(lnc[21])   (TensorE, 1 chunk each)
with idxg = y_comb if ys==1 else 20 (never matches -> mask row 0).
Host: loss = -(sum(A) + W0*sum(B) + W1*sum(C)) / (B*S)
"""

import sys

if '/opt/trn_rl_repo' not in sys.path:
    sys.path.insert(0, '/opt/trn_rl_repo')

import numpy as np
import ml_dtypes

import concourse.bass as bass
import concourse.bacc as bacc
import concourse.tile as tile
import concourse.mybir as mybir
from concourse.bass_utils import run_bass_kernel_spmd

F32 = mybir.dt.float32
BF16 = mybir.dt.bfloat16
FP8 = mybir.dt.float8e4
BF16_NP = ml_dtypes.bfloat16
FP8_NP = ml_dtypes.float8_e4m3fn

B, S, C = 64, 16384, 20
CE = C + 2                      # extended classes: +(1-ps)-gated, +ps-gated
W0, W1 = 0.51, 19.05
P = 128
N_CORES = 8
Tp = 128                        # tokens per partition per tile
NT = (B // N_CORES) * S // (P * Tp)   # tiles per core
IW = 64                         # iota inner period (dense run length)
AUXW = C * IW + NT * Tp         # iota block + all idxg tiles
MM = 512                        # matmul moving-free chunk (= psum bank f32)
ALU = mybir.AluOpType
AF = mybir.ActivationFunctionType


def _build(NT, Tp):
    FREE = Tp * CE              # full extended width
    CW = Tp * C                 # comb-classes width
    nc = bacc.Bacc("TRN2", target_bir_lowering=False, debug=False)

    comb_d = nc.dram_tensor("comb", [NT, P, FREE], FP8, kind="ExternalInput").ap()
    aux_d = nc.dram_tensor("aux", [P, AUXW], BF16, kind="ExternalInput").ap()
    out_d = nc.dram_tensor("out", [1, MM + 2 * Tp], F32, kind="ExternalOutput").ap()

    with tile.TileContext(nc) as tc:
        with (
            tc.tile_pool(name="const", bufs=1) as const_pool,
            tc.tile_pool(name="comb", bufs=1) as comb_pool,
            tc.tile_pool(name="lnc", bufs=2) as lnc_pool,
            tc.tile_pool(name="mask", bufs=2) as mask_pool,
            tc.tile_pool(name="prod", bufs=2) as prod_pool,
            tc.tile_pool(name="psum", bufs=1,
                         space=bass.MemorySpace.PSUM) as psum_pool,
        ):
            # warm the natural_log activation table while first DMAs run
            warm = const_pool.tile([P, 1], BF16)
            nc.vector.memset(warm[:], 1.0)
            nc.scalar.activation(warm[:], warm[:], AF.Ln)

            ones = const_pool.tile([P, 1], BF16)
            nc.vector.memset(ones[:], 1.0)

            # comb0 leads the queue (it gates the ActE Ln chain), then the
            # small aux (iota + all idxg), then the rest of the comb stream.
            comb_t0 = comb_pool.tile([P, FREE], FP8, tag="comb0")
            comb_ts = [comb_t0]
            nc.sync.dma_start(comb_t0[:], comb_d[0])

            aux_t = const_pool.tile([P, AUXW], BF16)
            nc.sync.dma_start(aux_t[:], aux_d[:])
            iota_v = aux_t[:, 0:C * IW].rearrange("p (c o t) -> p c o t",
                                                  c=C, o=1)

            for i in range(1, NT):
                comb_t = comb_pool.tile([P, FREE], FP8, tag=f"comb{i}")
                nc.sync.dma_start(comb_t[:], comb_d[i])
                comb_ts.append(comb_t)

            pA = psum_pool.tile([1, MM], F32, tag="pA")
            pB = psum_pool.tile([1, Tp], F32, tag="pB")
            pC = psum_pool.tile([1, Tp], F32, tag="pC")

            for i in range(NT):
                comb_t = comb_ts[i]
                off = C * IW + i * Tp
                idxg = aux_t[:, off:off + Tp]
                idxg_v = idxg.rearrange("p (o r t) -> p o r t", o=1, t=IW)

                lnc = lnc_pool.tile([P, FREE], BF16, tag="lnc")
                nc.scalar.activation(lnc[:], comb_t[:], AF.Ln)

                mask = mask_pool.tile([P, CW], BF16, tag="mask")
                mask_v = mask[:].rearrange("p (c r t) -> p c r t", c=C, t=IW)
                b_iota, b_idxg = bass.broadcast_tensor_aps(iota_v, idxg_v)
                nc.vector.tensor_tensor(mask_v, b_iota, b_idxg, ALU.is_equal)

                prod = prod_pool.tile([P, CW], BF16, tag="prod")
                nc.vector.tensor_tensor(prod[:], mask[:], lnc[:, 0:CW],
                                        ALU.mult)

                first, last = (i == 0), (i == NT - 1)
                for c in range(CW // MM):
                    nc.tensor.matmul(pA[:], ones[:],
                                     prod[:, c * MM:(c + 1) * MM],
                                     start=(first and c == 0),
                                     stop=(last and c == CW // MM - 1))
                nc.tensor.matmul(pB[:], ones[:], lnc[:, CW:CW + Tp],
                                 start=first, stop=last)
                nc.tensor.matmul(pC[:], ones[:], lnc[:, CW + Tp:FREE],
                                 start=first, stop=last)

            res_t = const_pool.tile([1, MM + 2 * Tp], F32)
            nc.scalar.copy(res_t[:, 0:MM], pA[:])
            nc.scalar.copy(res_t[:, MM:MM + Tp], pB[:])
            nc.scalar.copy(res_t[:, MM + Tp:MM + 2 * Tp], pC[:])
            nc.sync.dma_start(out_d[:], res_t[:])

    nc.compile()
    return nc


_NC_CACHE = {}


def make_in_maps(y_pred_stroke, y_pred_comb, y_stroke, y_comb):
    y_pred_stroke = np.asarray(y_pred_stroke, dtype=np.float32)
    y_pred_comb = np.asarray(y_pred_comb, dtype=np.float32)
    y_stroke = np.asarray(y_stroke, dtype=np.float32)
    y_comb = np.asarray(y_comb)
    FREE = Tp * CE
    Bc = B // N_CORES
    iota = np.repeat(np.arange(C, dtype=np.float32), IW)
    in_maps = []
    for c in range(N_CORES):
        sl = slice(c * Bc, (c + 1) * Bc)
        ys = np.ascontiguousarray(y_stroke[sl])[..., 0].reshape(-1)
        ps = np.ascontiguousarray(y_pred_stroke[sl])[..., 0].reshape(-1)
        yc = np.ascontiguousarray(y_comb[sl]).reshape(-1)
        pos = ys > 0.5
        comb = (np.maximum(np.ascontiguousarray(y_pred_comb[sl]), 2.0 ** -9)
                .reshape(NT, P, Tp, C)
                .transpose(0, 1, 3, 2))                     # [NT, P, C, Tp]
        q0 = np.where(pos, 1.0, 1.0 - ps).reshape(NT, P, 1, Tp)
        q1 = np.where(pos, ps, 1.0).reshape(NT, P, 1, Tp)
        comb_ext = np.concatenate([comb, q0, q1], axis=2).reshape(NT, P, FREE)
        idxg = np.where(pos, yc.astype(np.float32), 20.0)
        aux = np.empty((P, AUXW), dtype=np.float32)
        aux[:, 0:C * IW] = iota[None, :]
        aux[:, C * IW:] = (idxg.reshape(NT, P, Tp)
                           .transpose(1, 0, 2).reshape(P, NT * Tp))
        in_maps.append({
            "comb": np.ascontiguousarray(comb_ext).astype(FP8_NP),
            "aux": aux.astype(BF16_NP),
        })
    return in_maps


def kernel(y_pred_stroke, y_pred_comb, y_stroke, y_comb):
    key = (NT, Tp)
    if key not in _NC_CACHE:
        _NC_CACHE[key] = _build(NT, Tp)
    nc = _NC_CACHE[key]
    in_maps = make_in_maps(y_pred_stroke, y_pred_comb, y_stroke, y_comb)
    res = run_bass_kernel_spmd(nc, in_maps, list(range(N_CORES)))
    total = 0.0
    for r in res.results:
        o = r["out"].astype(np.float64).reshape(-1)
        total += (o[0:MM].sum() + W0 * o[MM:MM + Tp].sum()
                  + W1 * o[MM + Tp:].sum())
    return np.asarray([-total / (B * S)], dtype=np.float32)
